# revision 1
# baseline (speedup 1.0000x reference)
"""TRN2 Bass kernel for nn_DiffusionUNet_64 (moe_routing).

Computation per sample b:
    pooled = mean(x[b], HW)                       (CIN,)
    rw = softmax(router(pooled, time_emb[b]))     (E,)
    w_eff = sum_e rw[e] * weight[e]               (COUT, CIN, 3, 3)
    y[b] = conv2d(x[b], w_eff, pad=1)             (COUT, H, W)

Sharding: data-parallel over batch, 4 samples per core on 8 cores.
The conv runs as 9 shifted fp16 matmuls (fp32 PSUM accumulation), two
samples interleaved per offset so the PE consumes weight-DMA chunks as
they arrive. Expert mixing uses the delta identity
(softmax weights sum to 1): weff = W0 + sum_e s_e * (We - W0),
split across DVE/ACT/GpSimd. The router runs in fp32 on-device.
"""
import numpy as np

import concourse.bass as bass
import concourse.tile as tile
from concourse import bacc, mybir
from concourse.bass_utils import run_bass_kernel_spmd

F32 = mybir.dt.float32
F32R = mybir.dt.float32r
BF16 = mybir.dt.bfloat16
FP16 = mybir.dt.float16
WT_MODE = "fp16"
POOL_ON_ACT = False
WTDT = {"fp16": FP16, "bf16": BF16, "fp32": F32}[WT_MODE]

B, CIN, COUT, H, W = 32, 256, 256, 32, 32
E, TDIM, HID = 4, 256, 64
NCORES = 8
BLOC = B // NCORES          # 4 samples per core
NCH = CIN // 128            # 2 cin chunks
MCH = COUT // 128           # 2 cout chunks
HP, WP = H + 2, W + 2       # 34x34 padded
PIX = H * W                 # 1024
NPARAM = 528


def build_program(do_mix=True, do_conv=True):
    nc = bacc.Bacc("TRN2", target_bir_lowering=False, debug=False,
                   num_devices=NCORES)
    xp_d = nc.dram_tensor("xpad", [BLOC, 128, NCH, HP * WP], FP16,
                          kind="ExternalInput").ap()
    te_d = nc.dram_tensor("temb", [128, NCH, BLOC], F32, kind="ExternalInput").ap()
    wt_d = nc.dram_tensor("wt", [128, 9, NCH, E, COUT], WTDT,
                          kind="ExternalInput").ap()
    rp_d = nc.dram_tensor("rparams", [128, NPARAM], F32, kind="ExternalInput").ap()
    out_d = nc.dram_tensor("out", [BLOC, MCH, 128, PIX], F32,
                           kind="ExternalOutput").ap()
    rwsc_d = nc.dram_tensor("rwscratch", [BLOC, E], F32).ap()

    AF = mybir.ActivationFunctionType
    ALU = mybir.AluOpType

    with tile.TileContext(nc) as tc:
        with tc.tile_pool(name="persist", bufs=1) as pp, \
             tc.tile_pool(name="weff", bufs=3) as wp, \
             tc.tile_pool(name="work", bufs=4) as wk, \
             tc.tile_pool(name="rwork", bufs=4) as rwk, \
             tc.tile_pool(name="osb", bufs=4) as ob, \
             tc.tile_pool(name="ps", bufs=8, space="PSUM") as ps:

            # ---- persistent tiles + input DMAs (just-in-time order)
            rp = pp.tile([128, NPARAM], F32)
            te = pp.tile([128, NCH, BLOC], F32)
            nc.sync.dma_start(rp[:], rp_d[:])
            nc.sync.dma_start(te[:], te_d[:])

            xp = pp.tile([128, BLOC, NCH, HP * WP], FP16)
            wt = pp.tile([128, 9, NCH, E, COUT], WTDT)
            nc.sync.dma_start(xp[:, 0, 0], xp_d[0, :, 0])
            nc.sync.dma_start(xp[:, 0, 1], xp_d[0, :, 1])
            nc.sync.dma_start(xp[:, 1], xp_d[1])
            nc.sync.dma_start(xp[:, 2], xp_d[2])
            nc.sync.dma_start(xp[:, 3], xp_d[3])
            for o in range(9):
                nc.gpsimd.dma_start(wt[:, o:o + 1], wt_d[:, o:o + 1])

            ones1 = pp.tile([1, 128], F32)
            nc.vector.memset(ones1[:], 1.0)
            xm_pre = []
            for b in range(BLOC):
                xmt = pp.tile([HID + 1, 1], F32, name=f"xm_{b}")
                nc.vector.memset(xmt[HID:HID + 1, :], 1.0)
                xm_pre.append(xmt)

            # ---- routers (stage-major, fused DVE ops), emitted per PAIR so
            # late x2/x3 DMAs never head-of-line-block pair0's engine FIFOs
            pooled = [pp.tile([128, NCH], F32, name=f"pooled_{b}")
                      for b in range(BLOC)]
            rwbs = [None] * BLOC

            def rmm(tag, cols, rhs_fn, b):
                pt = ps.tile([HID, 1], F32, tag="ps8", name=f"{tag}_{b}")
                for c in range(NCH):
                    nc.tensor.matmul(pt[:], rp[:, cols + c * HID:cols + (c + 1) * HID],
                                     rhs_fn(c), start=(c == 0), stop=(c == NCH - 1))
                return pt

            pscr = pp.tile([128, HP * WP], F32)

            def emit_routers(bs):
                for b in bs:
                    if b == 0:
                        # ACT is idle earliest; per-chunk accum right after DMA
                        for c in range(NCH):
                            nc.scalar.activation(pscr[:], xp[:, 0, c],
                                                 AF.Identity,
                                                 accum_out=pooled[0][:, c:c + 1])
                    else:
                        nc.vector.tensor_reduce(pooled[b][:], xp[:, b],
                                                mybir.AxisListType.X, ALU.add)
                qs = {}
                for b in bs:
                    rq = rmm("rq", 0, lambda c: te[:, c, b:b + 1], b)
                    q = rwk.tile([HID, 1], F32, tag="qs", name=f"qs_{b}")
                    nc.vector.tensor_scalar_add(q[:], rq[:], rp[0:HID, 516:517])
                    qs[b] = q
                t1s = {}
                for b in bs:
                    rk = rmm("rk", 128, lambda c: pooled[b][:, c:c + 1], b)
                    t1 = rwk.tile([HID, 1], F32, tag="t1", name=f"t1_{b}")
                    nc.vector.scalar_tensor_tensor(t1[:], rk[:], rp[0:HID, 517:518],
                                                   qs[b][:], ALU.add, ALU.mult)
                    t1s[b] = t1
                attns = {}
                for b in bs:
                    attn = rwk.tile([HID, 1], F32, tag="attn", name=f"attn_{b}")
                    nc.scalar.activation(attn[:], t1s[b][:], AF.Sigmoid)
                    attns[b] = attn
                xas = {}
                for b in bs:
                    rv = rmm("rv", 256, lambda c: pooled[b][:, c:c + 1], b)
                    xa = rwk.tile([HID, 1], F32, tag="xa", name=f"xa_{b}")
                    nc.vector.scalar_tensor_tensor(xa[:], rv[:], rp[0:HID, 518:519],
                                                   attns[b][:], ALU.add, ALU.mult)
                    xas[b] = xa
                h1ss = {}
                for b in bs:
                    rh1 = ps.tile([HID, 1], F32, tag="ps8", name=f"rh1_{b}")
                    nc.tensor.matmul(rh1[:], rp[0:HID, 384:448], xas[b][:],
                                     start=True, stop=True)
                    h1s = rwk.tile([HID, 1], F32, tag="h1s", name=f"h1s_{b}")
                    nc.scalar.activation(h1s[:], rh1[:], AF.Silu,
                                         bias=rp[0:HID, 519:520])
                    h1ss[b] = h1s
                xms = {}
                for b in bs:
                    rh2 = ps.tile([HID, 1], F32, tag="ps8", name=f"rh2_{b}")
                    nc.tensor.matmul(rh2[:], rp[0:HID, 448:512], h1ss[b][:],
                                     start=True, stop=True)
                    xm = xm_pre[b]
                    nc.vector.scalar_tensor_tensor(xm[0:HID, :], rh2[:],
                                                   rp[0:HID, 520:521], xas[b][:],
                                                   ALU.add, ALU.add)
                    xms[b] = xm
                expss = {}
                for b in bs:
                    rl = ps.tile([1, E], F32, tag="ps8", name=f"rl_{b}")
                    nc.tensor.matmul(rl[:], xms[b][:], rp[0:HID + 1, 512:516],
                                     start=True, stop=True)
                    exps = rwk.tile([1, E], F32, tag="exps", name=f"exps_{b}")
                    nc.scalar.activation(exps[:], rl[:], AF.Exp)
                    expss[b] = exps
                for b in bs:
                    rwp = ps.tile([128, E], F32, tag="ps8", name=f"rwp_{b}")
                    nc.tensor.matmul(rwp[:], ones1[:], expss[b][:],
                                     start=True, stop=True)
                    ssum = rwk.tile([128, 1], F32, tag="ssum", name=f"ssum_{b}")
                    nc.vector.tensor_reduce(ssum[:], rwp[:], mybir.AxisListType.X,
                                            ALU.add)
                    srec = rwk.tile([128, 1], F32, tag="srec", name=f"srec_{b}")
                    nc.vector.reciprocal(srec[:], ssum[:])
                    rwb = pp.tile([128, E], F32, name=f"rwb_{b}")
                    nc.vector.tensor_scalar_mul(rwb[:], rwp[:], srec[:])
                    rwbs[b] = rwb

            def mix_weff(b, o):
                rwb = rwbs[b]
                wtile = wp.tile([128, NCH, COUT], FP16, tag=f"weff_{o}",
                                name=f"weff_{b}_{o}")
                if do_mix and (o + b) % 2 == 0:
                    acc = wk.tile([128, NCH, COUT], FP16, tag="maccv",
                                  name=f"acc_{b}_{o}")
                    nc.vector.scalar_tensor_tensor(acc[:], wt[:, o, :, 1],
                                                   rwb[:, 1:2], wt[:, o, :, 0],
                                                   ALU.mult, ALU.add)
                    nc.vector.scalar_tensor_tensor(acc[:], wt[:, o, :, 2],
                                                   rwb[:, 2:3], acc[:],
                                                   ALU.mult, ALU.add)
                    nc.vector.scalar_tensor_tensor(wtile[:], wt[:, o, :, 3],
                                                   rwb[:, 3:4], acc[:],
                                                   ALU.mult, ALU.add)
                elif do_mix:
                    p2 = wk.tile([128, NCH, COUT], FP16, tag="mact0",
                                 name=f"p2_{b}_{o}")
                    nc.scalar.activation(p2[:], wt[:, o, :, 2], AF.Identity,
                                         scale=rwb[:, 2:3])
                    p3 = wk.tile([128, NCH, COUT], FP16, tag="mact1",
                                 name=f"p3_{b}_{o}")
                    nc.scalar.activation(p3[:], wt[:, o, :, 3], AF.Identity,
                                         scale=rwb[:, 3:4])
                    a1 = wk.tile([128, NCH, COUT], FP16, tag="maccv",
                                 name=f"a1_{b}_{o}")
                    nc.vector.scalar_tensor_tensor(a1[:], wt[:, o, :, 1],
                                                   rwb[:, 1:2], wt[:, o, :, 0],
                                                   ALU.mult, ALU.add)
                    a2 = wk.tile([128, NCH, COUT], FP16, tag="maccp",
                                 name=f"a2_{b}_{o}")
                    nc.gpsimd.tensor_tensor(a2[:], p2[:], p3[:], ALU.add)
                    nc.vector.tensor_tensor(wtile[:], a1[:], a2[:], ALU.add)
                else:
                    nc.vector.tensor_copy(wtile[:], wt[:, o, :, 0])
                return wtile

            def conv_rhs(b, c, o, nh):
                kh, kw = divmod(o, 3)
                return xp[:, b, c].rearrange("p (h w) -> p h w", h=HP)[
                    :, kh + 16 * nh:kh + 16 * nh + 16, kw:kw + 32]

            # ---- pair 0: offset-outer (consume weight chunks as they land)
            emit_routers((0, 1, 2, 3))
            pair = (0, 1)
            psums = {}
            for b in pair:
                for m in range(MCH):
                    for nh in range(2):
                        psums[(b, m, nh)] = ps.tile(
                            [128, 512], F32, tag="ps8",
                            name=f"cps_{b}_{m}_{nh}")
            for o in range(9):
                for b in pair:
                    wtile = mix_weff(b, o)
                    if not do_conv:
                        continue
                    for c in range(NCH):
                        for m in range(MCH):
                            lhsT = wtile[:, c, m * 128:(m + 1) * 128]
                            for nh in range(2):
                                nc.tensor.matmul(
                                    psums[(b, m, nh)], lhsT, conv_rhs(b, c, o, nh),
                                    start=(o == 0 and c == 0),
                                    stop=(o == 8 and c == NCH - 1))
            for b in pair:
                for m in range(MCH):
                    osb = ob.tile([128, PIX], F32, tag=f"osb_{m}",
                                  name=f"osb_{b}_{m}")
                    for nh in range(2):
                        nc.scalar.copy(osb[:, nh * 512:(nh + 1) * 512],
                                       psums[(b, m, nh)][:])
                        nc.gpsimd.dma_start(
                            out_d[b, m][:, nh * 512:(nh + 1) * 512],
                            osb[:, nh * 512:(nh + 1) * 512])

            # ---- pair 1: weights resident; m-sequential groups so drains
            # overlap the remaining matmul stream
            weff1 = {}
            for b in (2, 3):
                for o in range(9):
                    weff1[(b, o)] = mix_weff(b, o)
            for b in (2, 3):
                for m in range(MCH):
                    osb = ob.tile([128, PIX], F32, tag=f"osb_{m}",
                                  name=f"osb_{b}_{m}")
                    for nh in range(2):
                        psum = ps.tile([128, 512], F32, tag="ps8",
                                       name=f"cps_{b}_{m}_{nh}")
                        first = True
                        for o in range(9):
                            for c in range(NCH):
                                nc.tensor.matmul(
                                    psum[:], weff1[(b, o)][:, c, m * 128:(m + 1) * 128],
                                    conv_rhs(b, c, o, nh), start=first,
                                    stop=(o == 8 and c == NCH - 1))
                                first = False
                        nc.scalar.copy(osb[:, nh * 512:(nh + 1) * 512], psum[:])
                        nc.gpsimd.dma_start(
                            out_d[b, m][:, nh * 512:(nh + 1) * 512],
                            osb[:, nh * 512:(nh + 1) * 512])
    nc.compile()
    return nc


_PROGRAM = None


def _get_program():
    global _PROGRAM
    if _PROGRAM is None:
        _PROGRAM = build_program()
    return _PROGRAM


def _prep_shared(weight, Wq, bq, Wk, bk, Wv, bv, Wm1, bm1, Wm2, bm2, Wc, bc):
    # wt[p, o, c, e, cout] = weight[e, cout, c*128+p, kh, kw]
    w = weight.transpose(2, 3, 4, 0, 1)                   # (CIN,3,3,E,COUT)
    w = w.reshape(NCH, 128, 3, 3, E, COUT).transpose(1, 2, 3, 0, 4, 5)
    wt = np.ascontiguousarray(w.reshape(128, 9, NCH, E, COUT), dtype=np.float32)
    # delta form: slot e>0 := W_e - W_0 (softmax weights sum to 1)
    wt[:, :, :, 1:] -= wt[:, :, :, 0:1]

    rp = np.zeros((128, NPARAM), dtype=np.float32)
    WqT = Wq.T.reshape(NCH, 128, HID)                     # [c,p,j]
    WkT = (Wk / float(PIX)).T.reshape(NCH, 128, HID)
    WvT = (Wv / float(PIX)).T.reshape(NCH, 128, HID)
    for c in range(NCH):
        rp[:, c * HID:(c + 1) * HID] = WqT[c]
        rp[:, 128 + c * HID:128 + (c + 1) * HID] = WkT[c]
        rp[:, 256 + c * HID:256 + (c + 1) * HID] = WvT[c]
    rp[0:HID, 384:448] = Wm1.T
    rp[0:HID, 448:512] = Wm2.T
    rp[0:HID, 512:516] = Wc.T
    rp[HID, 512:516] = bc
    rp[0:HID, 516] = bq
    rp[0:HID, 517] = bk
    rp[0:HID, 518] = bv
    rp[0:HID, 519] = bm1
    rp[0:HID, 520] = bm2
    return wt, rp


def kernel(x, time_emb, weight, Wq, bq, Wk, bk, Wv, bv, Wm1, bm1, Wm2, bm2,
           Wc, bc):
    x = np.asarray(x, dtype=np.float32)
    time_emb = np.asarray(time_emb, dtype=np.float32)
    wt, rp = _prep_shared(np.asarray(weight, np.float32),
                          np.asarray(Wq, np.float32), np.asarray(bq, np.float32),
                          np.asarray(Wk, np.float32), np.asarray(bk, np.float32),
                          np.asarray(Wv, np.float32), np.asarray(bv, np.float32),
                          np.asarray(Wm1, np.float32), np.asarray(bm1, np.float32),
                          np.asarray(Wm2, np.float32), np.asarray(bm2, np.float32),
                          np.asarray(Wc, np.float32), np.asarray(bc, np.float32))

    if WT_MODE == "fp16":
        wt_dev = wt.astype(np.float16)
    elif WT_MODE == "bf16":
        import ml_dtypes
        wt_dev = wt.astype(ml_dtypes.bfloat16)
    else:
        wt_dev = wt
    in_maps = []
    for i in range(NCORES):
        xl = x[i * BLOC:(i + 1) * BLOC]                   # (4,256,32,32)
        xr = xl.reshape(BLOC, NCH, 128, H, W).transpose(0, 2, 1, 3, 4).astype(np.float16)
        xpad = np.zeros((BLOC, 128, NCH, HP, WP), dtype=np.float16)
        xpad[:, :, :, 1:H + 1, 1:W + 1] = xr
        xpad = np.ascontiguousarray(xpad.reshape(BLOC, 128, NCH, HP * WP))

        tl = time_emb[i * BLOC:(i + 1) * BLOC]            # (4,256)
        te = np.ascontiguousarray(
            tl.T.reshape(NCH, 128, BLOC).transpose(1, 0, 2))

        in_maps.append({"xpad": xpad, "temb": te, "wt": wt_dev, "rparams": rp})

    nc = _get_program()
    res = run_bass_kernel_spmd(nc, in_maps, list(range(NCORES))).results

    y = np.empty((B, COUT, H, W), dtype=np.float32)
    for i in range(NCORES):
        y[i * BLOC:(i + 1) * BLOC] = res[i]["out"].reshape(BLOC, COUT, H, W)
    return y

